# revision 4
# baseline (speedup 1.0000x reference)
"""Trainium2 Bass kernel for the DualLoss nn.Module (v2).

Strategy
--------
dist[b,m,s,n] = ||P[b,m,s] - X[b,n,m]||^2 is computed TWICE as fp32r
(hardware full-rate fp32, ~13-bit effective mantissa) matmuls with K=5
contraction rows per m: {-2P_c | pp | 1} x {X_c | 1 | xx}.

Both layouts use 4x PE row-tiling (tile_position=(32j,0), K<=32 per row
group) so four matmuls run CONCURRENTLY in the 128x128 array:

  Layout A (per (b,mquad)):  4 banks [s=128, n-chunk 512], one m per
    row group j. d2 = min over n via dual-stream TT_MINRED custom DVE
    ops (PSUM + scalar-staged SBUF pairs).
  Layout B (per (b,nchunk)): 4 banks [n=128, (m,s) 512], four m per
    row group (K=20). d1 = segmented tensor_reduce over s (DVE banks
    0-2, GpSimd bank 3).

Batch (B=16) is data-parallel across the 8 NeuronCores (2 batches per
core). A and B steps interleave so the PE streams B matmuls while A
banks drain. Host applies the argsort / stick-breaking weighting and
superquadric area weighting in float64.
"""

import sys

for _p in ("/opt/trn_rl_repo", "/root/.axon_site", "/root/.axon_site/_ro/trn_rl_repo",
           "/root/.axon_site/_ro/pypackages"):
    if _p not in sys.path:
        sys.path.append(_p)

import numpy as np

import concourse.bass as bass
import concourse.tile as tile
from concourse import bacc, mybir
from concourse.bass_utils import run_bass_kernel_spmd
from concourse import dve_ops as _dve_ops
from concourse.dve_ops import DveOp as _DveOp
from concourse.dve_spec import (
    Spec as _Spec, Src0 as _Src0, Src1 as _Src1, C0 as _C0, AluOp as _AluOp,
    minn as _minn, lower as _lower, _has_src1,
)
from concourse.dve_uop import DveOpSpec as _DveOpSpec


def _register_dve_op(name, spec):
    """Register a custom DVE op at runtime (sha computed on the fly)."""
    if name in _dve_ops._SUB_OPCODE_FOR_NAME:
        return next(op for op in _dve_ops.OPS if op.name == name)
    row = _dve_ops._CUSTOM_DVE_ROW_BASE + len(_dve_ops.OPS)
    assert row < 0x20
    _dve_ops._SUB_OPCODE_FOR_NAME[name] = row
    shas = {}
    for ver in ("v3", "v4"):
        tmp = _DveOpSpec(name=name, opcode=row, uops=_lower(spec, ver=ver),
                         rd1_en=_has_src1(spec))
        shas[ver] = tmp.sha(ver)
    op = _DveOp(name, spec, subdim=False, uops_sha=shas)
    _dve_ops.OPS.append(op)
    _dve_ops.CUSTOM_DVE_SPECS[name] = spec
    return op


# out = min(in0, in1); accum_out = min(seed, min(out)) - consumes two fp32
# streams (one PSUM + one SBUF) per cycle: a 2x-throughput fused min-reduce.
TT_MINRED = _register_dve_op(
    "TT_MINRED_ANT",
    _Spec(
        body=_minn(_Src0, _Src1),
        accum=_AluOp.MIN,
        accum_init=_C0,
        reference=lambda in0, in1, s0, s1, imm2: np.minimum(
            in0.astype(np.float32), in1),
    ),
)

F32 = mybir.dt.float32
F32R = mybir.dt.float32r
ALU = mybir.AluOpType

B, N, M, S = 16, 2048, 16, 128
CORES = 8
BPC = B // CORES          # batches per core = 2
NCHUNK = N // 128         # 16
NQUAD = 8                 # m-quads per core: 2 b x 4 quads
FOUR_PI = 4.0 * np.pi

_PROGRAM = None
LAST_RESULTS = None       # for test.py to read exec_time_ns


def _build_program(gps_bank=False):
    nc = bacc.Bacc("TRN2", target_bir_lowering=False, debug=False)

    # A: per quad q (b=q//4): 4 sub-tensors [5,128]/[5,2048] per row group j
    a_stat_d = nc.dram_tensor("a_stat", [NQUAD, 4, 5, 128], F32R,
                              kind="ExternalInput").ap()
    a_mov_d = nc.dram_tensor("a_mov", [NQUAD, 4, 5, N], F32R,
                             kind="ExternalInput").ap()
    # B: stationary per (b, nchunk): 4 x [20, 128]; moving per b: 4 x [20, 512]
    b_stat_d = nc.dram_tensor("b_stat", [BPC, NCHUNK, 4, 20, 128], F32R,
                              kind="ExternalInput").ap()
    b_mov_d = nc.dram_tensor("b_mov", [BPC, 4, 20, 512], F32R,
                             kind="ExternalInput").ap()
    # outputs: d2 halves (host folds h), d1 per (b, nchunk, m)
    d2o_d = nc.dram_tensor("d2o", [128, NQUAD, 4, 2], F32, kind="ExternalOutput").ap()
    d1o_d = nc.dram_tensor("d1o", [128, BPC, NCHUNK, M], F32,
                           kind="ExternalOutput").ap()

    from contextlib import ExitStack

    with tile.TileContext(nc) as tc, ExitStack() as ctx:
        const = ctx.enter_context(tc.tile_pool(name="const", bufs=1))
        p_ast = ctx.enter_context(tc.tile_pool(name="ast", bufs=2))
        p_amv = ctx.enter_context(tc.tile_pool(name="amv", bufs=2))
        p_bst = ctx.enter_context(tc.tile_pool(name="bst", bufs=3))
        p_psA = ctx.enter_context(tc.tile_pool(name="psA", bufs=1, space="PSUM"))
        p_psB = ctx.enter_context(tc.tile_pool(name="psB", bufs=1, space="PSUM"))
        p_stg = ctx.enter_context(tc.tile_pool(name="stg", bufs=2))
        p_scr = ctx.enter_context(tc.tile_pool(name="scr", bufs=2))

        # B moving operands: resident
        bmv = []
        for b in range(BPC):
            t = const.tile([128, 512], F32R, tag=f"bmv{b}", name=f"bmv{b}")
            for j in range(4):
                nc.sync.dma_start(out=t[32*j:32*j+20, :], in_=b_mov_d[b, j])
            bmv.append(t)

        # output accumulators (written in slices, DMA'd once)
        d2t = const.tile([128, NQUAD, 4, 2], F32, tag="d2t")
        d1all = const.tile([128, BPC, NCHUNK, M], F32, tag="d1all")

        for i in range(32):
            # ---------------- layout A step: quad q, chunk c ----------------
            q, c = i // 4, i % 4
            if c == 0:
                ast = p_ast.tile([128, 128], F32R, tag="ast", name=f"ast{q}")
                amv = p_amv.tile([128, N], F32R, tag="amv", name=f"amv{q}")
                for j in range(4):
                    nc.sync.dma_start(out=ast[32*j:32*j+5, :], in_=a_stat_d[q, j])
                    nc.sync.dma_start(out=amv[32*j:32*j+5, :], in_=a_mov_d[q, j])
                cur_ast, cur_amv = ast, amv
                stg = p_stg.tile([128, 4, 512], F32, tag="stg", name=f"stg{q}")
            pA = []
            for j in range(4):
                t = p_psA.tile([128, 512], F32, tag=f"pA{j}", name=f"pA{q}_{c}_{j}")
                nc.tensor.matmul(
                    t[:], lhsT=cur_ast[32*j:32*j+5, :],
                    rhs=cur_amv[32*j:32*j+5, 512*c:512*(c+1)],
                    start=True, stop=True, tile_position=(32*j, 0))
                pA.append(t)
            if c % 2 == 0:
                # stage even chunks to SBUF for the dual-stream fold
                for j in range(4):
                    nc.scalar.copy(stg[:, j, :], pA[j][:])
            else:
                # odd chunks: fold PSUM bank with the staged even chunk
                h = c // 2
                scr = p_scr.tile([128, 512], F32, tag="scr", name=f"scr{q}_{c}")
                for j in range(4):
                    nc.vector._custom_dve(
                        TT_MINRED, out=scr[:], in0=pA[j][:], in1=stg[:, j, :],
                        s0=3.0e38, accum_out=d2t[:, q, j, h:h+1])

            # ---------------- layout B step: batch bb, nchunk cc ------------
            bb, cc = i // 16, i % 16
            bst = p_bst.tile([128, 128], F32R, tag="bst", name=f"bst{i}")
            for j in range(4):
                nc.sync.dma_start(out=bst[32*j:32*j+20, :], in_=b_stat_d[bb, cc, j])
            pB = []
            for j in range(4):
                t = p_psB.tile([128, 512], F32, tag=f"pB{j}", name=f"pB{i}_{j}")
                nc.tensor.matmul(
                    t[:], lhsT=bst[32*j:32*j+20, :], rhs=bmv[bb][32*j:32*j+20, :],
                    start=True, stop=True, tile_position=(32*j, 0))
                pB.append(t)
            for j in range(4):
                eng = nc.gpsimd if (gps_bank and j == 3) else nc.vector
                eng.tensor_reduce(
                    out=d1all[:, bb, cc, 4*j:4*j+4],
                    in_=pB[j][:].rearrange("p (m s) -> p m s", m=4),
                    axis=mybir.AxisListType.X, op=ALU.min)

        nc.sync.dma_start(out=d2o_d, in_=d2t[:])
        nc.sync.dma_start(out=d1o_d, in_=d1all[:])

    nc.compile()
    return nc


def _get_program():
    global _PROGRAM
    if _PROGRAM is None:
        _PROGRAM = _build_program()
    return _PROGRAM


def _make_in_maps(pcl, prim):
    """Host-side packing of the fp32 A/B operands (per core)."""
    X = pcl            # (B, N, M, 3) f32
    P = prim           # (B, M, S, 3) f32
    xx = np.einsum("bnmc,bnmc->bnm", X.astype(np.float64), X.astype(np.float64))
    pp = np.einsum("bmsc,bmsc->bms", P.astype(np.float64), P.astype(np.float64))
    xx = xx.astype(np.float32)     # (B, N, M)
    pp = pp.astype(np.float32)     # (B, M, S)

    in_maps = []
    for core in range(CORES):
        bsl = slice(BPC * core, BPC * (core + 1))
        Xc, Pc = X[bsl], P[bsl]
        xxc, ppc = xx[bsl], pp[bsl]

        a_stat = np.empty((NQUAD, 4, 5, 128), np.float32)
        a_mov = np.empty((NQUAD, 4, 5, N), np.float32)
        for q in range(NQUAD):
            b, qq = q // 4, q % 4
            for j in range(4):
                m = 4 * qq + j
                a_stat[q, j, 0:3] = -2.0 * Pc[b, m].T          # (3, S)
                a_stat[q, j, 3] = ppc[b, m]
                a_stat[q, j, 4] = 1.0
                a_mov[q, j, 0:3] = Xc[b, :, m, :].T            # (3, N)
                a_mov[q, j, 3] = 1.0
                a_mov[q, j, 4] = xxc[b, :, m]

        b_stat = np.empty((BPC, NCHUNK, 4, 20, 128), np.float32)
        b_mov = np.zeros((BPC, 4, 20, 512), np.float32)
        for b in range(BPC):
            for j in range(4):
                for t in range(4):
                    m = 4 * j + t
                    r0, cs = 5 * t, slice(128 * t, 128 * (t + 1))
                    b_mov[b, j, r0:r0+3, cs] = Pc[b, m].T      # (3, S)
                    b_mov[b, j, r0+3, cs] = ppc[b, m]
                    b_mov[b, j, r0+4, cs] = 1.0
                    for ccn in range(NCHUNK):
                        nsl = slice(128 * ccn, 128 * (ccn + 1))
                        b_stat[b, ccn, j, r0:r0+3] = -2.0 * Xc[b, nsl, m, :].T
                        b_stat[b, ccn, j, r0+3] = 1.0
                        b_stat[b, ccn, j, r0+4] = xxc[b, nsl, m]
        in_maps.append({"a_stat": a_stat, "a_mov": a_mov,
                        "b_stat": b_stat, "b_mov": b_mov})
    return in_maps


def kernel(pcl_transformed, primitive_points, size, probs, _trace=False):
    global LAST_RESULTS
    pcl = np.asarray(pcl_transformed, dtype=np.float32)
    prim = np.asarray(primitive_points, dtype=np.float32)
    size = np.asarray(size, dtype=np.float32)
    probs = np.asarray(probs, dtype=np.float32)

    nc = _get_program()
    in_maps = _make_in_maps(pcl, prim)
    res = run_bass_kernel_spmd(nc, in_maps, list(range(CORES)), trace=_trace)
    LAST_RESULTS = res

    # ---- host-side final reductions (float64) ----
    d2min = np.empty((B, M, S), np.float64)
    d1 = np.empty((B, N, M), np.float64)
    for core in range(CORES):
        d2o = res.results[core]["d2o"].astype(np.float64)    # [128, 8, 4, 2]
        d2q = d2o.min(axis=3)                                # [128(s), 8, 4]
        for q in range(NQUAD):
            b, qq = q // 4, q % 4
            for j in range(4):
                d2min[BPC * core + b, 4 * qq + j] = d2q[:, q, j]
        d1o = res.results[core]["d1o"].astype(np.float64)    # [128, 2, 16, 16]
        d1[BPC * core: BPC * (core + 1)] = (
            d1o.transpose(1, 2, 0, 3).reshape(BPC, N, M))

    # stick-breaking weights, vectorized reference-style (argsort + cumprod)
    p64v = probs.astype(np.float64)
    d1f = d1.reshape(B * N, M)
    order = np.argsort(d1f, axis=1, kind="stable")
    ps = np.take_along_axis(np.repeat(p64v, N, axis=0), order, axis=1)
    ncp = np.cumprod(1.0 - ps, axis=1)
    ncp = np.concatenate([np.ones((B * N, 1)), ncp[:, :-1]], axis=1)
    p2p_sum = float((np.take_along_axis(d1f, order, axis=1) * ps * ncp).sum())

    d2 = np.where(d2min >= 1e30, 0.0, d2min)                 # (B, M, S)

    s0 = size[..., 0].astype(np.float64)
    s1 = size[..., 1].astype(np.float64)
    s2 = size[..., 2].astype(np.float64)
    area = FOUR_PI * ((s0 * s1) ** 1.6 / 3 + (s0 * s2) ** 1.6 / 3
                      + (s1 * s2) ** 1.6 / 3) ** 0.625
    area = M * area / area.sum(axis=-1, keepdims=True)

    prim_to_pcl = float(
        (d2.mean(axis=-1) * probs.astype(np.float64) * area).sum() / (B * M))
    pcl_to_prim = float(p2p_sum / (B * N))

    total = np.float32(pcl_to_prim + prim_to_pcl)
    return (total,
            np.float32(pcl_to_prim),
            np.float32(prim_to_pcl),
            np.float32(0.0))
